# revision 16
# baseline (speedup 1.0000x reference)
"""Trainium2 kernel for nn_Band_49022756717118 (band-split -> per-band MLP -> overlap-add).

Key observation: the reference pipeline (gather bands -> pre_w matmul -> post_w
matmul -> mask -> scatter-add -> OLA divide) has NO nonlinearity, so the whole
module is one linear operator on the flattened (freq, channel) axis:

    out[(f',c'), (b,t)] = sum_{(f,c)} A[(f',c'), (f,c)] * x[(f,c), (b,t)]

A is [2050, 2050], banded with |r'-r| <= 2*(W-1)+1 = 59 (bands are contiguous
frequency ranges of width <= 30 overlapping by ~50%).  Padding to 17 blocks of
128 rows, A is block-tridiagonal, so each output block needs at most 3
[128,128] @ [128,N] matmuls accumulated in PSUM.

Distribution: pure data-parallel over batch B=16 -> 2 batches per core, the
small folded weight blocks replicated on every core.  No collectives.

Device work per core: DMA x (bf16), 98 TensorE matmuls, PSUM->SBUF cast
copies, DMA out (bf16).  Host does layout/sharding and the tiny weight fold
(64 matmuls of [60,128]@[128,60]) plus the final f32 upcast.
"""

import os

import numpy as np
import ml_dtypes

import bass_rust
import concourse.bass as bass
import concourse.mybir as mybir
import concourse.tile as tile
from concourse.bass_utils import run_bass_kernel_spmd
from concourse.vector_clock import ScopedClock, VectorClock


def _patch_tile_drain():
    """walrus on this target accepts at most ONE sync wait per instruction, but
    TileContext's kernel-tail drain carries a wait for every active proc.
    Split them: one single-wait NOP on the sync engine per proc, then drain."""
    if getattr(tile.TileContext, "_drain_patched", False):
        return

    def _drain_and_barrier(self, tick_clock, wait_clock):
        nc = self.nc
        gc = tick_clock.global_clock
        vals = [int(s) for s in repr(gc).split("[")[1].split("]")[0].split(",")]
        for proc, tick in enumerate(vals):
            if tick <= 0:
                continue
            single = [0] * len(vals)
            single[proc] = tick
            n = nc.sync.nop(nofuse=True)
            wait_clock.add_sem_waits(
                n.ins, ScopedClock({None: VectorClock(single)})
            )
        # the single-wait NOPs above run in-order on the SP stream, so the
        # drain itself needs no waits of its own
        nc.sync.drain()
        nc.all_engine_barrier()
        assert self.sems is not None
        popped = nc._tile_sem_poison_stack.pop()
        assert popped is self._sem_poison
        nc.clear_and_free_semaphores(list(self.sems.allocated().values()))
        nc.all_engine_barrier()

    tile.TileContext._drain_and_barrier = _drain_and_barrier
    tile.TileContext._drain_patched = True


_patch_tile_drain()

# Problem constants (hardcoded per harness contract)
B, F, T, C = 16, 1025, 512, 2
R = F * C                 # 2050 flattened (f, c) rows
P = 128                   # partitions per block
NB = (R + P - 1) // P     # 17 row blocks
RP = NB * P               # 2176 padded rows
NCORES = 8
BPC = B // NCORES         # batches per core
N = BPC * T               # 1024 columns per core
CHUNK = 512               # matmul free-dim chunk (1 PSUM bank in f32)

BF16 = mybir.dt.bfloat16
F32 = mybir.dt.float32

# (o, d) pairs in device iteration order; host packs weight blocks identically.
BLOCK_PAIRS = [(o, d) for o in range(NB) for d in (-1, 0, 1) if 0 <= o + d < NB]
NW = len(BLOCK_PAIRS)     # 49

LAST_EXEC_TIME_NS = None
LAST_RESULTS = None

_nc_cache = None


def _ensure_ntff_hook():
    """Register the axon NTFF profiling hook if the image lacks antenv.axon_hooks."""
    try:
        from antenv.axon_hooks import get_axon_ntff_profile_hook  # noqa: F401

        return True
    except ImportError:
        pass
    try:
        import sys
        import types

        import antenv
        import trn_agent_boot.trn_boot as tb

        hook = tb._ntff_profile_via_ctypes("/opt/axon/libaxon_pjrt.so")
        if hook is None:
            return False
        mod = types.ModuleType("antenv.axon_hooks")
        mod._hook = hook
        mod.get_axon_ntff_profile_hook = lambda: mod._hook

        def _set(h):
            mod._hook = h

        mod.set_axon_ntff_profile_hook = _set
        sys.modules["antenv.axon_hooks"] = mod
        antenv.axon_hooks = mod
        return True
    except Exception:
        return False


def _build_nc():
    """Build the SPMD Bass graph (identical on all 8 cores)."""
    nc = bass.Bass()
    # partition-major DRAM layouts: every DMA is a plain 2D slice (no rearrange)
    x_d = nc.declare_dram_parameter("x", [P, NB * N], BF16, isOutput=False)
    w_d = nc.declare_dram_parameter("w", [P, NW * P], BF16, isOutput=False)
    o_d = nc.declare_dram_parameter("out", [P, NB * N], BF16, isOutput=True)

    # 8 output groups -> 8 out-DMAs, one per SW-DGE queue (a DMA instruction
    # can carry only ONE sync wait, so each group uses a single copy engine
    # and a fresh queue)
    groups = [[0, 1, 2]] + [[i, i + 1] for i in range(3, NB, 2)]
    assert sum(len(g) for g in groups) == NB and len(groups) == 8

    with tile.TileContext(nc) as tc:
        with (
            tc.tile_pool(name="xp", bufs=NB) as xp,
            tc.tile_pool(name="wp", bufs=1) as wp,
            tc.tile_pool(name="op", bufs=len(groups)) as op,
            tc.tile_pool(name="ps", bufs=8, space="PSUM") as ps,
        ):
            wt = wp.tile([P, NW * P], BF16)
            nc.sync.dma_start(wt[:], w_d[:])

            xtiles = []
            for o in range(NB):
                xt = xp.tile([P, N], BF16)
                nc.sync.dma_start(xt[:], x_d[:, o * N : (o + 1) * N])
                xtiles.append(xt)

            widx = {pair: i for i, pair in enumerate(BLOCK_PAIRS)}
            for gi, group in enumerate(groups):
                ot = op.tile([P, len(group) * N], BF16)
                use_act = gi % 3 == 2
                for oi, o in enumerate(group):
                    ds = [d for d in (-1, 0, 1) if 0 <= o + d < NB]
                    for ci in range(N // CHUNK):
                        pt = ps.tile([P, CHUNK], F32)
                        for j, d in enumerate(ds):
                            wi = widx[(o, d)]
                            nc.tensor.matmul(
                                pt[:],
                                wt[:, wi * P : (wi + 1) * P],
                                xtiles[o + d][:, ci * CHUNK : (ci + 1) * CHUNK],
                                start=(j == 0),
                                stop=(j == len(ds) - 1),
                            )
                        dst = ot[:, oi * N + ci * CHUNK : oi * N + (ci + 1) * CHUNK]
                        if use_act:
                            nc.scalar.copy(dst, pt[:])
                        else:
                            nc.vector.tensor_copy(dst, pt[:])
                nc.gpsimd.dma_start(
                    o_d[:, group[0] * N : (group[0] + len(group)) * N], ot[:]
                )
    return nc


def _fold_operator(f_idxes, mask, ola, pre_w, pre_b, post_w, post_b):
    """Fold the whole reference pipeline into banded matrix A + constant."""
    K, WC, D = pre_w.shape
    W = WC // C
    fi = f_idxes.reshape(K, W).astype(np.int64)
    mk = mask.reshape(K, W)

    A = np.zeros((R, R), dtype=np.float64)
    const = np.zeros(R, dtype=np.float64)
    for k in range(K):
        M = pre_w[k].astype(np.float64) @ post_w[k].astype(np.float64)
        cvec = pre_b[k].astype(np.float64) @ post_w[k].astype(np.float64) + post_b[k]
        pos = (fi[k][:, None] * C + np.arange(C)[None, :]).reshape(-1)
        mflat = np.repeat(mk[k], C)
        valid = mflat > 0
        pv = pos[valid]
        Mv = (M * mflat[:, None] * mflat[None, :])[np.ix_(valid, valid)]
        A[np.ix_(pv, pv)] += Mv.T  # A[r_out, r_in] += M[i_in, i_out]
        const[pv] += (cvec * mflat)[valid]
    ola2 = np.repeat(ola.astype(np.float64), C)
    A /= ola2[:, None]
    const /= ola2
    return A, const


def kernel(x, f_idxes, mask, ola_window, pre_w, pre_b, post_w, post_b):
    global LAST_EXEC_TIME_NS, LAST_RESULTS, _nc_cache

    x = np.asarray(x, dtype=np.float32)
    f_idxes = np.asarray(f_idxes)
    mask = np.asarray(mask, dtype=np.float32)
    ola_window = np.asarray(ola_window, dtype=np.float32)
    pre_w = np.asarray(pre_w, dtype=np.float32)
    pre_b = np.asarray(pre_b, dtype=np.float32)
    post_w = np.asarray(post_w, dtype=np.float32)
    post_b = np.asarray(post_b, dtype=np.float32)

    A, const = _fold_operator(f_idxes, mask, ola_window, pre_w, pre_b, post_w, post_b)

    # lhsT block for (o, d): w[k_in_local, m_out_local] = A[128o+m, 128(o+d)+k]
    Ap = np.zeros((RP, RP), dtype=np.float64)
    Ap[:R, :R] = A
    wflat = np.empty((P, NW * P), dtype=ml_dtypes.bfloat16)
    for i, (o, d) in enumerate(BLOCK_PAIRS):
        blk = Ap[o * P : (o + 1) * P, (o + d) * P : (o + d + 1) * P].T
        wflat[:, i * P : (i + 1) * P] = blk.astype(np.float32).astype(ml_dtypes.bfloat16)
    if os.environ.get("KERNEL_DEBUG_IDENTITY", "0") == "1":
        for i, (o, d) in enumerate(BLOCK_PAIRS):
            blk = np.eye(P, dtype=np.float32) if d == 0 else np.zeros((P, P), np.float32)
            wflat[:, i * P : (i + 1) * P] = blk.astype(ml_dtypes.bfloat16)

    # x -> [r=(f,c), b, t], pad, shard over batch, then partition-major
    # device layout [P, NB*N]: x_dev[p, o*N + n] = xr[o*P + p, b, t]
    xr = np.zeros((RP, B, T), dtype=ml_dtypes.bfloat16)
    xr[:R] = x.transpose(1, 3, 0, 2).reshape(R, B, T).astype(ml_dtypes.bfloat16)
    in_maps = []
    for cid in range(NCORES):
        xc = xr[:, cid * BPC : (cid + 1) * BPC, :].reshape(NB, P, N)
        xc = np.ascontiguousarray(xc.transpose(1, 0, 2).reshape(P, NB * N))
        in_maps.append({"x": xc, "w": wflat})

    if _nc_cache is None:
        _nc_cache = _build_nc()
    nc = _nc_cache

    trace = os.environ.get("KERNEL_TRACE", "0") == "1" and _ensure_ntff_hook()
    if trace:
        # skip the slow artifact upload; we only want exec_time_ns + local trace
        import concourse.bass_utils as _bu

        _bu.upload_artifacts = lambda tmpdir: tmpdir
    res = run_bass_kernel_spmd(nc, in_maps, core_ids=list(range(NCORES)), trace=trace)
    LAST_EXEC_TIME_NS = res.exec_time_ns
    LAST_RESULTS = res

    # gather + unshard: [P, NB*N] bf16 per core -> [B,F,T,C] f32
    outr = np.empty((R, B, T), dtype=np.float32)
    for cid in range(NCORES):
        oc = np.asarray(res.results[cid]["out"], dtype=np.float32)
        oc = oc.reshape(P, NB, N).transpose(1, 0, 2).reshape(RP, BPC, T)
        outr[:, cid * BPC : (cid + 1) * BPC, :] = oc[:R]
    out = outr.reshape(F, C, B, T).transpose(2, 0, 3, 1)
    if np.any(const != 0.0):  # biases are zero in this problem, but stay general
        out = out + const.reshape(F, C).astype(np.float32)[None, :, None, :]
    return np.ascontiguousarray(out)


# revision 18
# speedup vs baseline: 1.0343x; 1.0343x over previous
"""Trainium2 kernel for nn_Band_49022756717118 (band-split -> per-band MLP -> overlap-add).

Key observation: the reference pipeline (gather bands -> pre_w matmul -> post_w
matmul -> mask -> scatter-add -> OLA divide) has NO nonlinearity, so the whole
module is one linear operator on the flattened (freq, channel) axis:

    out[(f',c'), (b,t)] = sum_{(f,c)} A[(f',c'), (f,c)] * x[(f,c), (b,t)]

A is [2050, 2050], banded with |r'-r| <= 2*(W-1)+1 = 59 (bands are contiguous
frequency ranges of width <= 30 overlapping by ~50%).  Padding to 17 blocks of
128 rows, A is block-tridiagonal, so each output block needs at most 3
[128,128] @ [128,N] matmuls accumulated in PSUM.

Distribution: pure data-parallel over batch B=16 -> 2 batches per core, the
small folded weight blocks replicated on every core.  No collectives.

Device work per core: DMA x (bf16), 98 TensorE matmuls, PSUM->SBUF cast
copies, DMA out (bf16).  Host does layout/sharding and the tiny weight fold
(64 matmuls of [60,128]@[128,60]) plus the final f32 upcast.
"""

import os

import numpy as np
import ml_dtypes

import bass_rust
import concourse.bass as bass
import concourse.mybir as mybir
import concourse.tile as tile
from concourse.bass_utils import run_bass_kernel_spmd
from concourse.vector_clock import ScopedClock, VectorClock


def _patch_tile_drain():
    """walrus on this target accepts at most ONE sync wait per instruction, but
    TileContext's kernel-tail drain carries a wait for every active proc.
    Split them: one single-wait NOP on the sync engine per proc, then drain."""
    if getattr(tile.TileContext, "_drain_patched", False):
        return

    def _drain_and_barrier(self, tick_clock, wait_clock):
        nc = self.nc
        gc = tick_clock.global_clock
        vals = [int(s) for s in repr(gc).split("[")[1].split("]")[0].split(",")]
        for proc, tick in enumerate(vals):
            if tick <= 0:
                continue
            single = [0] * len(vals)
            single[proc] = tick
            n = nc.sync.nop(nofuse=True)
            wait_clock.add_sem_waits(
                n.ins, ScopedClock({None: VectorClock(single)})
            )
        # the single-wait NOPs above run in-order on the SP stream, so the
        # drain itself needs no waits of its own
        nc.sync.drain()
        nc.all_engine_barrier()
        assert self.sems is not None
        popped = nc._tile_sem_poison_stack.pop()
        assert popped is self._sem_poison
        nc.clear_and_free_semaphores(list(self.sems.allocated().values()))
        nc.all_engine_barrier()

    tile.TileContext._drain_and_barrier = _drain_and_barrier
    tile.TileContext._drain_patched = True


_patch_tile_drain()

# Problem constants (hardcoded per harness contract)
B, F, T, C = 16, 1025, 512, 2
R = F * C                 # 2050 flattened (f, c) rows
P = 128                   # partitions per block
NB = (R + P - 1) // P     # 17 row blocks
RP = NB * P               # 2176 padded rows
NCORES = 8
BPC = B // NCORES         # batches per core
N = BPC * T               # 1024 columns per core
CHUNK = 512               # matmul free-dim chunk (1 PSUM bank in f32)

BF16 = mybir.dt.bfloat16
F32 = mybir.dt.float32

# (o, d) pairs in device iteration order; host packs weight blocks identically.
BLOCK_PAIRS = [(o, d) for o in range(NB) for d in (-1, 0, 1) if 0 <= o + d < NB]
NW = len(BLOCK_PAIRS)     # 49

LAST_EXEC_TIME_NS = None
LAST_RESULTS = None

_nc_cache = None


def _ensure_ntff_hook():
    """Register the axon NTFF profiling hook if the image lacks antenv.axon_hooks."""
    try:
        from antenv.axon_hooks import get_axon_ntff_profile_hook  # noqa: F401

        return True
    except ImportError:
        pass
    try:
        import sys
        import types

        import antenv
        import trn_agent_boot.trn_boot as tb

        hook = tb._ntff_profile_via_ctypes("/opt/axon/libaxon_pjrt.so")
        if hook is None:
            return False
        mod = types.ModuleType("antenv.axon_hooks")
        mod._hook = hook
        mod.get_axon_ntff_profile_hook = lambda: mod._hook

        def _set(h):
            mod._hook = h

        mod.set_axon_ntff_profile_hook = _set
        sys.modules["antenv.axon_hooks"] = mod
        antenv.axon_hooks = mod
        return True
    except Exception:
        return False


def _build_nc():
    """Build the SPMD Bass graph (identical on all 8 cores)."""
    nc = bass.Bass()
    # partition-major DRAM layouts: every DMA is a plain 2D slice (no rearrange)
    x_d = nc.declare_dram_parameter("x", [P, NB * N], BF16, isOutput=False)
    w_d = nc.declare_dram_parameter("w", [P, NW * P], BF16, isOutput=False)
    o_d = nc.declare_dram_parameter("out", [P, NB * N], BF16, isOutput=True)

    # 8 output groups -> 8 out-DMAs on gpsimd, one per SW-DGE queue (a DMA
    # instruction can carry only ONE sync wait, so each group uses a single
    # copy engine and a fresh queue; everything else goes via sync/HW-DGE)
    ogroups = [[0, 1, 2]] + [[i, i + 1] for i in range(3, NB, 2)]
    assert sum(len(g) for g in ogroups) == NB and len(ogroups) == 8
    # x-input chunks of 3 blocks (fewer DMA issues, fine-enough dep granularity)
    xgroups = [[0, 1, 2], [3, 4, 5], [6, 7, 8], [9, 10, 11], [12, 13, 14], [15, 16]]
    xg_of = {o: (g, gi.index(o)) for g, gi in enumerate(xgroups) for o in gi}
    widx = {pair: i for i, pair in enumerate(BLOCK_PAIRS)}
    # weight DMA per ogroup: pairs for an ogroup are contiguous in BLOCK_PAIRS
    wranges = []
    for group in ogroups:
        idxs = [widx[(o, d)] for o in group for d in (-1, 0, 1) if (o, d) in widx]
        assert idxs == list(range(idxs[0], idxs[-1] + 1))
        wranges.append((idxs[0], idxs[-1] + 1))

    with tile.TileContext(nc) as tc:
        with (
            tc.tile_pool(name="xp", bufs=len(xgroups)) as xp,
            tc.tile_pool(name="wp", bufs=len(ogroups)) as wp,
            tc.tile_pool(name="op", bufs=len(ogroups)) as op,
            tc.tile_pool(name="ps", bufs=4, space="PSUM") as ps,
        ):
            # interleave weight-group and x-chunk loads so the first output
            # group's operands arrive as early as possible
            wtiles = [None] * len(ogroups)
            xtiles = [None] * len(xgroups)
            for i in range(max(len(ogroups), len(xgroups))):
                if i < len(ogroups):
                    lo, hi = wranges[i]
                    wt = wp.tile([P, (hi - lo) * P], BF16)
                    nc.sync.dma_start(wt[:], w_d[:, lo * P : hi * P])
                    wtiles[i] = wt
                if i < len(xgroups):
                    blocks = xgroups[i]
                    xt = xp.tile([P, len(blocks) * N], BF16)
                    nc.sync.dma_start(
                        xt[:], x_d[:, blocks[0] * N : (blocks[0] + len(blocks)) * N]
                    )
                    xtiles[i] = xt

            def x_ap(o, cs, ce):
                g, li = xg_of[o]
                return xtiles[g][:, li * N + cs : li * N + ce]

            last_mm = {}
            copies = {}
            u = 0  # global output-block counter (psum slot = u % ps.bufs)
            for gi, group in enumerate(ogroups):
                ot = op.tile([P, len(group) * N], BF16)
                use_act = gi % 3 == 2
                wt = wtiles[gi]
                wlo = wranges[gi][0]
                for oi, o in enumerate(group):
                    ds = [d for d in (-1, 0, 1) if 0 <= o + d < NB]
                    pt = ps.tile([P, N], F32)
                    if u >= 4 and u - 1 in last_mm:
                        # hoist the PSUM-slot WAR (copy[u-4] must drain before
                        # this block's start=True matmul) onto the previous
                        # block's last matmul, which carries no other waits --
                        # walrus allows only ONE sync wait per instruction
                        tile.add_dep_helper(
                            last_mm[u - 1].ins,
                            copies[u - 4].ins,
                            sync=True,
                            reason="psum WAR prehoist",
                        )
                    mm = None
                    for ci in range(N // CHUNK):
                        for j, d in enumerate(ds):
                            wl = widx[(o, d)] - wlo
                            mm = nc.tensor.matmul(
                                pt[:, ci * CHUNK : (ci + 1) * CHUNK],
                                wt[:, wl * P : (wl + 1) * P],
                                x_ap(o + d, ci * CHUNK, (ci + 1) * CHUNK),
                                start=(j == 0),
                                stop=(j == len(ds) - 1),
                            )
                    last_mm[u] = mm
                    dst = ot[:, oi * N : (oi + 1) * N]
                    if use_act:
                        copies[u] = nc.scalar.copy(dst, pt[:])
                    else:
                        copies[u] = nc.vector.tensor_copy(dst, pt[:])
                    u += 1
                nc.gpsimd.dma_start(
                    o_d[:, group[0] * N : (group[0] + len(group)) * N], ot[:]
                )
    return nc


def _fold_operator(f_idxes, mask, ola, pre_w, pre_b, post_w, post_b):
    """Fold the whole reference pipeline into banded matrix A + constant."""
    K, WC, D = pre_w.shape
    W = WC // C
    fi = f_idxes.reshape(K, W).astype(np.int64)
    mk = mask.reshape(K, W)

    A = np.zeros((R, R), dtype=np.float64)
    const = np.zeros(R, dtype=np.float64)
    for k in range(K):
        M = pre_w[k].astype(np.float64) @ post_w[k].astype(np.float64)
        cvec = pre_b[k].astype(np.float64) @ post_w[k].astype(np.float64) + post_b[k]
        pos = (fi[k][:, None] * C + np.arange(C)[None, :]).reshape(-1)
        mflat = np.repeat(mk[k], C)
        valid = mflat > 0
        pv = pos[valid]
        Mv = (M * mflat[:, None] * mflat[None, :])[np.ix_(valid, valid)]
        A[np.ix_(pv, pv)] += Mv.T  # A[r_out, r_in] += M[i_in, i_out]
        const[pv] += (cvec * mflat)[valid]
    ola2 = np.repeat(ola.astype(np.float64), C)
    A /= ola2[:, None]
    const /= ola2
    return A, const


def kernel(x, f_idxes, mask, ola_window, pre_w, pre_b, post_w, post_b):
    global LAST_EXEC_TIME_NS, LAST_RESULTS, _nc_cache

    x = np.asarray(x, dtype=np.float32)
    f_idxes = np.asarray(f_idxes)
    mask = np.asarray(mask, dtype=np.float32)
    ola_window = np.asarray(ola_window, dtype=np.float32)
    pre_w = np.asarray(pre_w, dtype=np.float32)
    pre_b = np.asarray(pre_b, dtype=np.float32)
    post_w = np.asarray(post_w, dtype=np.float32)
    post_b = np.asarray(post_b, dtype=np.float32)

    A, const = _fold_operator(f_idxes, mask, ola_window, pre_w, pre_b, post_w, post_b)

    # lhsT block for (o, d): w[k_in_local, m_out_local] = A[128o+m, 128(o+d)+k]
    Ap = np.zeros((RP, RP), dtype=np.float64)
    Ap[:R, :R] = A
    wflat = np.empty((P, NW * P), dtype=ml_dtypes.bfloat16)
    for i, (o, d) in enumerate(BLOCK_PAIRS):
        blk = Ap[o * P : (o + 1) * P, (o + d) * P : (o + d + 1) * P].T
        wflat[:, i * P : (i + 1) * P] = blk.astype(np.float32).astype(ml_dtypes.bfloat16)
    if os.environ.get("KERNEL_DEBUG_IDENTITY", "0") == "1":
        for i, (o, d) in enumerate(BLOCK_PAIRS):
            blk = np.eye(P, dtype=np.float32) if d == 0 else np.zeros((P, P), np.float32)
            wflat[:, i * P : (i + 1) * P] = blk.astype(ml_dtypes.bfloat16)

    # x -> [r=(f,c), b, t], pad, shard over batch, then partition-major
    # device layout [P, NB*N]: x_dev[p, o*N + n] = xr[o*P + p, b, t]
    xr = np.zeros((RP, B, T), dtype=ml_dtypes.bfloat16)
    xr[:R] = x.transpose(1, 3, 0, 2).reshape(R, B, T).astype(ml_dtypes.bfloat16)
    in_maps = []
    for cid in range(NCORES):
        xc = xr[:, cid * BPC : (cid + 1) * BPC, :].reshape(NB, P, N)
        xc = np.ascontiguousarray(xc.transpose(1, 0, 2).reshape(P, NB * N))
        in_maps.append({"x": xc, "w": wflat})

    if _nc_cache is None:
        _nc_cache = _build_nc()
    nc = _nc_cache

    trace = os.environ.get("KERNEL_TRACE", "0") == "1" and _ensure_ntff_hook()
    if trace:
        # skip the slow artifact upload; we only want exec_time_ns + local trace
        import concourse.bass_utils as _bu

        _bu.upload_artifacts = lambda tmpdir: tmpdir
    res = run_bass_kernel_spmd(nc, in_maps, core_ids=list(range(NCORES)), trace=trace)
    LAST_EXEC_TIME_NS = res.exec_time_ns
    LAST_RESULTS = res

    # gather + unshard: [P, NB*N] bf16 per core -> [B,F,T,C] f32
    outr = np.empty((R, B, T), dtype=np.float32)
    for cid in range(NCORES):
        oc = np.asarray(res.results[cid]["out"], dtype=np.float32)
        oc = oc.reshape(P, NB, N).transpose(1, 0, 2).reshape(RP, BPC, T)
        outr[:, cid * BPC : (cid + 1) * BPC, :] = oc[:R]
    out = outr.reshape(F, C, B, T).transpose(2, 0, 3, 1)
    if np.any(const != 0.0):  # biases are zero in this problem, but stay general
        out = out + const.reshape(F, C).astype(np.float32)[None, :, None, :]
    return np.ascontiguousarray(out)


# revision 21
# speedup vs baseline: 1.0963x; 1.0600x over previous
"""Trainium2 kernel for nn_Band_49022756717118 (band-split -> per-band MLP -> overlap-add).

Key observation: the reference pipeline (gather bands -> pre_w matmul -> post_w
matmul -> mask -> scatter-add -> OLA divide) has NO nonlinearity, so the whole
module is one linear operator on the flattened (freq, channel) axis:

    out[(f',c'), (b,t)] = sum_{(f,c)} A[(f',c'), (f,c)] * x[(f,c), (b,t)]

A is [2050, 2050], banded with |r'-r| <= 2*(W-1)+1 = 59 (bands are contiguous
frequency ranges of width <= 30 overlapping by ~50%).  Padding to 17 blocks of
128 rows, A is block-tridiagonal, so each output block needs at most 3
[128,128] @ [128,N] matmuls accumulated in PSUM.

Distribution: pure data-parallel over batch B=16 -> 2 batches per core, the
small folded weight blocks replicated on every core.  No collectives.

Device work per core: DMA x (bf16), 98 TensorE matmuls, PSUM->SBUF cast
copies, DMA out (bf16).  Host does layout/sharding and the tiny weight fold
(64 matmuls of [60,128]@[128,60]) plus the final f32 upcast.
"""

import os

import numpy as np
import ml_dtypes

import bass_rust
import concourse.bass as bass
import concourse.mybir as mybir
import concourse.tile as tile
from concourse.bass_utils import run_bass_kernel_spmd
from concourse.vector_clock import ScopedClock, VectorClock


def _patch_tile_drain():
    """walrus on this target accepts at most ONE sync wait per instruction, but
    TileContext's kernel-tail drain carries a wait for every active proc.
    Split them: one single-wait NOP on the sync engine per proc, then drain."""
    if getattr(tile.TileContext, "_drain_patched", False):
        return

    def _drain_and_barrier(self, tick_clock, wait_clock):
        nc = self.nc
        gc = tick_clock.global_clock
        vals = [int(s) for s in repr(gc).split("[")[1].split("]")[0].split(",")]
        for proc, tick in enumerate(vals):
            if tick <= 0:
                continue
            single = [0] * len(vals)
            single[proc] = tick
            n = nc.sync.nop(nofuse=True)
            wait_clock.add_sem_waits(
                n.ins, ScopedClock({None: VectorClock(single)})
            )
        # the single-wait NOPs above run in-order on the SP stream, so the
        # drain itself needs no waits of its own
        nc.sync.drain()
        nc.all_engine_barrier()
        assert self.sems is not None
        popped = nc._tile_sem_poison_stack.pop()
        assert popped is self._sem_poison
        nc.clear_and_free_semaphores(list(self.sems.allocated().values()))
        nc.all_engine_barrier()

    tile.TileContext._drain_and_barrier = _drain_and_barrier
    tile.TileContext._drain_patched = True


_patch_tile_drain()

# Problem constants (hardcoded per harness contract)
B, F, T, C = 16, 1025, 512, 2
R = F * C                 # 2050 flattened (f, c) rows
P = 128                   # partitions per block
NB = (R + P - 1) // P     # 17 row blocks
RP = NB * P               # 2176 padded rows
NCORES = 8
BPC = B // NCORES         # batches per core
N = BPC * T               # 1024 columns per core
CHUNK = 512               # matmul free-dim chunk (1 PSUM bank in f32)

BF16 = mybir.dt.bfloat16
F32 = mybir.dt.float32

# (o, d) pairs in device iteration order; host packs weight blocks identically.
BLOCK_PAIRS = [(o, d) for o in range(NB) for d in (-1, 0, 1) if 0 <= o + d < NB]
NW = len(BLOCK_PAIRS)     # 49

LAST_EXEC_TIME_NS = None
LAST_RESULTS = None

_nc_cache = None


def _ensure_ntff_hook():
    """Register the axon NTFF profiling hook if the image lacks antenv.axon_hooks."""
    try:
        from antenv.axon_hooks import get_axon_ntff_profile_hook  # noqa: F401

        return True
    except ImportError:
        pass
    try:
        import sys
        import types

        import antenv
        import trn_agent_boot.trn_boot as tb

        hook = tb._ntff_profile_via_ctypes("/opt/axon/libaxon_pjrt.so")
        if hook is None:
            return False
        mod = types.ModuleType("antenv.axon_hooks")
        mod._hook = hook
        mod.get_axon_ntff_profile_hook = lambda: mod._hook

        def _set(h):
            mod._hook = h

        mod.set_axon_ntff_profile_hook = _set
        sys.modules["antenv.axon_hooks"] = mod
        antenv.axon_hooks = mod
        return True
    except Exception:
        return False


def _build_nc():
    """Build the SPMD Bass graph (identical on all 8 cores)."""
    nc = bass.Bass()
    # partition-major DRAM layouts: every DMA is a plain 2D slice (no rearrange)
    x_d = nc.declare_dram_parameter("x", [P, NB * N], BF16, isOutput=False)
    w_d = nc.declare_dram_parameter("w", [P, NW * P], BF16, isOutput=False)
    o_d = nc.declare_dram_parameter("out", [P, NB * N], BF16, isOutput=True)

    # 8 output groups -> 8 out-DMAs on gpsimd, one per SW-DGE queue (a DMA
    # instruction can carry only ONE sync wait, so each group uses a single
    # copy engine and a fresh queue; everything else goes via sync/HW-DGE)
    ogroups = [[0, 1, 2]] + [[i, i + 1] for i in range(3, NB, 2)]
    assert sum(len(g) for g in ogroups) == NB and len(ogroups) == 8
    # x-input chunks (fewer DMA issues, fine-enough dep granularity); first is
    # small so the TensorE can start as early as possible
    xgroups = [[0, 1], [2, 3, 4], [5, 6, 7], [8, 9, 10], [11, 12, 13], [14, 15, 16]]
    xg_of = {o: (g, gi.index(o)) for g, gi in enumerate(xgroups) for o in gi}
    widx = {pair: i for i, pair in enumerate(BLOCK_PAIRS)}
    # weight DMA per ogroup: pairs for an ogroup are contiguous in BLOCK_PAIRS
    wranges = []
    for group in ogroups:
        idxs = [widx[(o, d)] for o in group for d in (-1, 0, 1) if (o, d) in widx]
        assert idxs == list(range(idxs[0], idxs[-1] + 1))
        wranges.append((idxs[0], idxs[-1] + 1))

    with tile.TileContext(nc) as tc:
        with (
            tc.tile_pool(name="xp", bufs=len(xgroups)) as xp,
            tc.tile_pool(name="wp", bufs=len(ogroups)) as wp,
            tc.tile_pool(name="op", bufs=len(ogroups)) as op,
            tc.tile_pool(name="ps", bufs=4, space="PSUM") as ps,
        ):
            # interleave weight-group and x-chunk loads so the first output
            # group's operands arrive as early as possible
            wtiles = [None] * len(ogroups)
            xtiles = [None] * len(xgroups)
            for i in range(max(len(ogroups), len(xgroups))):
                if i < len(ogroups):
                    lo, hi = wranges[i]
                    wt = wp.tile([P, (hi - lo) * P], BF16)
                    nc.sync.dma_start(wt[:], w_d[:, lo * P : hi * P])
                    wtiles[i] = wt
                if i < len(xgroups):
                    blocks = xgroups[i]
                    xt = xp.tile([P, len(blocks) * N], BF16)
                    nc.sync.dma_start(
                        xt[:], x_d[:, blocks[0] * N : (blocks[0] + len(blocks)) * N]
                    )
                    xtiles[i] = xt

            def x_ap(o, cs, ce):
                g, li = xg_of[o]
                return xtiles[g][:, li * N + cs : li * N + ce]

            last_mm = {}
            copies = {}
            u = 0  # global output-block counter (psum slot = u % ps.bufs)
            for gi, group in enumerate(ogroups):
                ot = op.tile([P, len(group) * N], BF16)
                use_act = gi % 2 == 1
                wt = wtiles[gi]
                wlo = wranges[gi][0]
                for oi, o in enumerate(group):
                    ds = [d for d in (-1, 0, 1) if 0 <= o + d < NB]
                    pt = ps.tile([P, N], F32)
                    if u >= 4 and u - 1 in last_mm:
                        # hoist the PSUM-slot WAR (copy[u-4] must drain before
                        # this block's start=True matmul) onto the previous
                        # block's last matmul, which carries no other waits --
                        # walrus allows only ONE sync wait per instruction
                        tile.add_dep_helper(
                            last_mm[u - 1].ins,
                            copies[u - 4].ins,
                            sync=True,
                            reason="psum WAR prehoist",
                        )
                    # diagonal block: full-array matmul, clears PSUM (start=True).
                    # off-diagonal blocks live in disjoint 64x64 array quadrants
                    # (corner -1: contract rows 64:128 -> out 0:64; corner +1:
                    # contract rows 0:64 -> out 64:128) so the two corner
                    # matmuls execute concurrently on TensorE.
                    mm = None
                    H = P // 2
                    for ci in range(N // CHUNK):
                        wl = widx[(o, 0)] - wlo
                        nc.tensor.matmul(
                            pt[:, ci * CHUNK : (ci + 1) * CHUNK],
                            wt[:, wl * P : (wl + 1) * P],
                            x_ap(o, ci * CHUNK, (ci + 1) * CHUNK),
                            start=True,
                            stop=False,
                            skip_group_check=True,
                        )
                    for ci in range(N // CHUNK):
                        for d in ds:
                            if d == 0:
                                continue
                            wl = widx[(o, d)] - wlo
                            wb = wt[:, wl * P : (wl + 1) * P]
                            if d == -1:
                                lhsT = wb[H:P, 0:H]
                                rhs = x_ap(o - 1, ci * CHUNK, (ci + 1) * CHUNK)[H:P, :]
                                out_ap = pt[0:H, ci * CHUNK : (ci + 1) * CHUNK]
                            else:
                                lhsT = wb[0:H, H:P]
                                rhs = x_ap(o + 1, ci * CHUNK, (ci + 1) * CHUNK)[0:H, :]
                                out_ap = pt[H:P, ci * CHUNK : (ci + 1) * CHUNK]
                            mm = nc.tensor.matmul(
                                out_ap,
                                lhsT,
                                rhs,
                                start=False,
                                stop=True,
                                skip_group_check=True,
                            )
                    last_mm[u] = mm
                    dst = ot[:, oi * N : (oi + 1) * N]
                    if use_act:
                        copies[u] = nc.scalar.copy(dst, pt[:])
                    else:
                        copies[u] = nc.vector.tensor_copy(dst, pt[:])
                    u += 1
                nc.gpsimd.dma_start(
                    o_d[:, group[0] * N : (group[0] + len(group)) * N], ot[:]
                )
    return nc


def _fold_operator(f_idxes, mask, ola, pre_w, pre_b, post_w, post_b):
    """Fold the whole reference pipeline into banded matrix A + constant."""
    K, WC, D = pre_w.shape
    W = WC // C
    fi = f_idxes.reshape(K, W).astype(np.int64)
    mk = mask.reshape(K, W)

    A = np.zeros((R, R), dtype=np.float64)
    const = np.zeros(R, dtype=np.float64)
    for k in range(K):
        M = pre_w[k].astype(np.float64) @ post_w[k].astype(np.float64)
        cvec = pre_b[k].astype(np.float64) @ post_w[k].astype(np.float64) + post_b[k]
        pos = (fi[k][:, None] * C + np.arange(C)[None, :]).reshape(-1)
        mflat = np.repeat(mk[k], C)
        valid = mflat > 0
        pv = pos[valid]
        Mv = (M * mflat[:, None] * mflat[None, :])[np.ix_(valid, valid)]
        A[np.ix_(pv, pv)] += Mv.T  # A[r_out, r_in] += M[i_in, i_out]
        const[pv] += (cvec * mflat)[valid]
    ola2 = np.repeat(ola.astype(np.float64), C)
    A /= ola2[:, None]
    const /= ola2
    return A, const


def kernel(x, f_idxes, mask, ola_window, pre_w, pre_b, post_w, post_b):
    global LAST_EXEC_TIME_NS, LAST_RESULTS, _nc_cache

    x = np.asarray(x, dtype=np.float32)
    f_idxes = np.asarray(f_idxes)
    mask = np.asarray(mask, dtype=np.float32)
    ola_window = np.asarray(ola_window, dtype=np.float32)
    pre_w = np.asarray(pre_w, dtype=np.float32)
    pre_b = np.asarray(pre_b, dtype=np.float32)
    post_w = np.asarray(post_w, dtype=np.float32)
    post_b = np.asarray(post_b, dtype=np.float32)

    A, const = _fold_operator(f_idxes, mask, ola_window, pre_w, pre_b, post_w, post_b)

    # lhsT block for (o, d): w[k_in_local, m_out_local] = A[128o+m, 128(o+d)+k]
    Ap = np.zeros((RP, RP), dtype=np.float64)
    Ap[:R, :R] = A
    wflat = np.empty((P, NW * P), dtype=ml_dtypes.bfloat16)
    for i, (o, d) in enumerate(BLOCK_PAIRS):
        blk = Ap[o * P : (o + 1) * P, (o + d) * P : (o + d + 1) * P].T
        wflat[:, i * P : (i + 1) * P] = blk.astype(np.float32).astype(ml_dtypes.bfloat16)
    if os.environ.get("KERNEL_DEBUG_IDENTITY", "0") == "1":
        for i, (o, d) in enumerate(BLOCK_PAIRS):
            blk = np.eye(P, dtype=np.float32) if d == 0 else np.zeros((P, P), np.float32)
            wflat[:, i * P : (i + 1) * P] = blk.astype(ml_dtypes.bfloat16)

    # x -> [r=(f,c), b, t], pad, shard over batch, then partition-major
    # device layout [P, NB*N]: x_dev[p, o*N + n] = xr[o*P + p, b, t]
    xr = np.zeros((RP, B, T), dtype=ml_dtypes.bfloat16)
    xr[:R] = x.transpose(1, 3, 0, 2).reshape(R, B, T).astype(ml_dtypes.bfloat16)
    in_maps = []
    for cid in range(NCORES):
        xc = xr[:, cid * BPC : (cid + 1) * BPC, :].reshape(NB, P, N)
        xc = np.ascontiguousarray(xc.transpose(1, 0, 2).reshape(P, NB * N))
        in_maps.append({"x": xc, "w": wflat})

    if _nc_cache is None:
        _nc_cache = _build_nc()
    nc = _nc_cache

    trace = os.environ.get("KERNEL_TRACE", "0") == "1" and _ensure_ntff_hook()
    if trace:
        # skip the slow artifact upload; we only want exec_time_ns + local trace
        import concourse.bass_utils as _bu

        _bu.upload_artifacts = lambda tmpdir: tmpdir
    res = run_bass_kernel_spmd(nc, in_maps, core_ids=list(range(NCORES)), trace=trace)
    LAST_EXEC_TIME_NS = res.exec_time_ns
    LAST_RESULTS = res

    # gather + unshard: [P, NB*N] bf16 per core -> [B,F,T,C] f32
    outr = np.empty((R, B, T), dtype=np.float32)
    for cid in range(NCORES):
        oc = np.asarray(res.results[cid]["out"], dtype=np.float32)
        oc = oc.reshape(P, NB, N).transpose(1, 0, 2).reshape(RP, BPC, T)
        outr[:, cid * BPC : (cid + 1) * BPC, :] = oc[:R]
    out = outr.reshape(F, C, B, T).transpose(2, 0, 3, 1)
    if np.any(const != 0.0):  # biases are zero in this problem, but stay general
        out = out + const.reshape(F, C).astype(np.float32)[None, :, None, :]
    return np.ascontiguousarray(out)
